# revision 16
# baseline (speedup 1.0000x reference)
"""Contrastive loss (SimCLR-style, B=1024, emb [1024,128,128]) on 8 TRN2 cores.

Strategy (v4): K-sharded upper-triangle gram + block ReduceScatter.

Host prep per core c: x chunk (m-slice of 16) pre-transposed / fp8-quantized in
DoubleRow layout x[k, n, (s, r)] = fp8(emb[r, 16c + 2k + s, n]) exactly as the
data-parallel hint's local shard, plus the per-(n, r) normalization scale
scale8 = 64/sqrt(128*ssq) (fp8) and small constant selector masks.

Device per core:
  1. rn = x * scale8 in place (fp8) on DVE, hi r-half first then lo in two
     descending 512-col chunks, so the PE can start early and the wide row
     tiles unlock progressively.
  2. Upper-triangle gram: row tiles in order [15..8, 7..0], partial sim block
     row [128, (16-i)*128] f32 on PE (fp8 DoubleRow, K=256/instr) accumulated
     over the core's 8 K-tiles; PSUM -> bf16 (ACT copy) -> DRAM as [128,128]
     blocks in production order (136 blocks of the 16x16 upper triangle).
  3. Three block-aligned ReduceScatters (48+48+40 blocks) sum the partial
     blocks across cores; each core ends up owning 17 whole summed blocks.
  4. Loss: per owned block (i,j): E = exp(2*sim), ACT accum -> row sums
     (rows of tile i); PE matmul E^T @ onehot(j) -> column sums (rows of
     tile j, zero mask for i==j); positives from the diagonals of the 8
     pair blocks (i, i+8) via an eye-mask reduce. Per-core row-sum vector
     P [128,16] and positive partials are either finished on device (P
     AllGather + log) or shipped to the host (HOST_FINISH).
Host: loss = (sum_r log(sum_c P_c - e^2) - 2*sum_c pos_c) / 2048.
"""

import numpy as np
import ml_dtypes

import concourse.bacc as bacc
import concourse.mybir as mybir
import concourse.tile as tile
from concourse import bass_utils

F32 = mybir.dt.float32
BF16 = mybir.dt.bfloat16
FP8 = mybir.dt.float8e4
AF = mybir.ActivationFunctionType
ALU = mybir.AluOpType
PM = mybir.MatmulPerfMode

B = 1024
R = 2 * B            # 2048 rows
NCORES = 8
KTILES = 8           # DoubleRow K-tiles per core (256 K each)
NT = 16              # 128-row tiles of sim
S = 64.0             # fp8 prescale; sim comes out x S^2
INV_T_S2 = 2.0 / (S * S)   # 1/TEMP / S^2
E2 = float(np.exp(2.0))    # exp(self-sim / TEMP), exact constant
N_WARM = 16
HOST_FINISH = True

# Upper-triangle blocks in production order: small hi tiles first, then the
# wide tiles widest-last so ReduceScatter chunks materialize early.
PROD_TILES = list(range(NT - 1, 7, -1)) + list(range(0, 8))
BLOCKS = [(i, j) for i in PROD_TILES for j in range(i, NT)]   # 136
CH_SIZES = [72, 64]
CH_CUM = [0, 72, 136]
NB = [n // NCORES for n in CH_SIZES]          # owned blocks/chunk: [6, 6, 5]
NSLOT = sum(NB)                               # 17
NCH = len(CH_SIZES)

# global production index of block (i, i)
_G0 = {}
_g = 0
for _i in PROD_TILES:
    _G0[_i] = _g
    _g += NT - _i

_CACHE = {}


def _core_slots(c):
    """Global block ids owned by core c, in slot order."""
    out = []
    for q, nb in enumerate(NB):
        out.extend(range(CH_CUM[q] + c * nb, CH_CUM[q] + (c + 1) * nb))
    return out


def _build_nc():
    if "nc" in _CACHE:
        return _CACHE["nc"]
    nc = bacc.Bacc("TRN2", target_bir_lowering=False, debug=False,
                   num_devices=NCORES)

    x = nc.dram_tensor("x", [KTILES, 128, 2 * R], FP8, kind="ExternalInput")
    scale8_d = nc.dram_tensor("scale8", [128, R], FP8, kind="ExternalInput")
    selrow_d = nc.dram_tensor("selrow", [128, NSLOT * NT], BF16,
                              kind="ExternalInput")
    selcol_d = nc.dram_tensor("selcol", [128, NSLOT * NT], BF16,
                              kind="ExternalInput")
    pairsel_d = nc.dram_tensor("pairsel", [128, NSLOT], BF16,
                               kind="ExternalInput")
    eye_d = nc.dram_tensor("eye", [128, 128], BF16, kind="ExternalInput")
    logmask_d = nc.dram_tensor("logmask", [128, NT], BF16,
                               kind="ExternalInput")
    if HOST_FINISH:
        y = nc.dram_tensor("y", [128, NT + 1], F32, kind="ExternalOutput")
    else:
        y = nc.dram_tensor("y", [1, 2], F32, kind="ExternalOutput")

    cc_fl_in = nc.dram_tensor("cc_fl_in", [1, 128], BF16)
    cc_fl_out = nc.dram_tensor("cc_fl_out", [NCORES, 128], BF16,
                               addr_space="Shared")
    cc_fl_out2 = nc.dram_tensor("cc_fl_out2", [NCORES, 128], BF16,
                                addr_space="Shared")
    cc_tri_in = [nc.dram_tensor(f"cc_tri_in{q}", [CH_SIZES[q] * 128, 128],
                                BF16) for q in range(NCH)]
    cc_tri_out = [nc.dram_tensor(f"cc_tri_out{q}", [NB[q] * 128, 128], BF16)
                  for q in range(NCH)]
    cc_p_in = nc.dram_tensor("cc_p_in", [128, NT], F32)
    cc_p_out = nc.dram_tensor("cc_p_out", [NCORES * 128, NT], F32,
                              addr_space="Shared")
    grp = [list(range(NCORES))]

    with tile.TileContext(nc) as tc:
        with tc.tile_pool(name="x8", bufs=KTILES) as px8, \
             tc.tile_pool(name="simsb", bufs=4) as psim, \
             tc.tile_pool(name="slab", bufs=2) as pslab, \
             tc.tile_pool(name="scr", bufs=3) as pscr, \
             tc.tile_pool(name="pers", bufs=1) as pers, \
             tc.tile_pool(name="ps", bufs=2, space="PSUM") as pps:

            # ---- t0 DVE: warmup fodder + small constants ----
            junk8 = pers.tile([128, 512], FP8, tag="junk8")
            nc.vector.memset(junk8[:], 0.25)
            junkA = pers.tile([128, 16], F32, tag="junkA")
            nc.vector.memset(junkA[:], 1.0)
            ones = pers.tile([128, 1], F32, tag="ones")
            nc.vector.memset(ones[:], 1.0)
            P_sb = pers.tile([128, NT], F32, tag="P_sb")
            nc.vector.memset(P_sb[:], 0.0)
            negE2 = pers.tile([128, 1], F32, tag="negE2")
            nc.vector.memset(negE2[:], -E2)
            # ACT table preload: exp set covers exp/ln/copy/square
            junkB = pers.tile([128, 16], F32, tag="junkB")
            nc.scalar.activation(junkB[:], junkA[:], AF.Exp)

            # ---- PE warmup ----
            jv = junk8[:].rearrange("p (two n) -> p two n", two=2)
            ps_w = pps.tile([128, R], F32, tag="ps")
            for w in range(N_WARM):
                nc.tensor.matmul(ps_w[:, 0:256], jv[:, :, 0:128],
                                 jv[:, :, 0:256],
                                 start=(w == 0), stop=(w == N_WARM - 1),
                                 perf_mode=PM.DoubleRow)

            # ---- x DMAs: hi halves on SP, lo halves on GP queue ----
            xb = [px8.tile([128, 2 * R], FP8, tag="x8", name=f"xb{k}")
                  for k in range(KTILES)]
            xv_d = [x[k].rearrange("p (s r) -> p s r", s=2)
                    for k in range(KTILES)]
            xv_s = [xb[k][:].rearrange("p (s r) -> p s r", s=2)
                    for k in range(KTILES)]
            scale8 = pers.tile([128, R], FP8, tag="scale8")
            nc.sync.dma_start(scale8[:], scale8_d[:])
            for k in range(KTILES):
                nc.sync.dma_start(xv_s[k][:, :, B:R], xv_d[k][:, :, B:R])
            # lo halves on the ACT queue: keeps the gpsimd queue free so the
            # first collective triggers (and the CC barrier starts) at t~0
            for k in range(KTILES):
                nc.scalar.dma_start(xv_s[k][:, :, 0:B], xv_d[k][:, :, 0:B])

            # ---- masks on SP (contiguous, fast) ----
            selrow_sb = pers.tile([128, NSLOT * NT], BF16, tag="selrow")
            nc.sync.dma_start(selrow_sb[:], selrow_d[:])
            selcol_sb = pers.tile([128, NSLOT * NT], BF16, tag="selcol")
            nc.sync.dma_start(selcol_sb[:], selcol_d[:])
            pairsel_sb = pers.tile([128, NSLOT], BF16, tag="pairsel")
            nc.sync.dma_start(pairsel_sb[:], pairsel_d[:])
            eye_sb = pers.tile([128, 128], BF16, tag="eye")
            nc.sync.dma_start(eye_sb[:], eye_d[:])
            logmask_sb = pers.tile([128, NT], BF16, tag="logmask")
            nc.sync.dma_start(logmask_sb[:], logmask_d[:])

            # ---- normalize in place on DVE: hi half, then lo descending ----
            for k in range(KTILES):
                for s in range(2):
                    sl = xb[k][:, s * R + B: s * R + R]
                    nc.vector.tensor_tensor(sl, sl, scale8[:, B:R], ALU.mult)
            for c0 in (0, 512):
                for k in range(KTILES):
                    for s in range(2):
                        sl = xb[k][:, s * R + c0: s * R + c0 + 512]
                        nc.vector.tensor_tensor(sl, sl,
                                                scale8[:, c0:c0 + 512],
                                                ALU.mult)

            # ---- upper-triangle gram in production order ----
            for i in PROD_TILES:
                w_i = (NT - i) * 128
                ps = pps.tile([128, R], F32, tag="ps")
                for k in range(KTILES):
                    lhsT = xv_s[k][:, :, i * 128:(i + 1) * 128]
                    # chunks aligned to the 512-col PSUM bank grid (a matmul
                    # dst must not cross a bank boundary); descending so the
                    # hi r-half is consumed first
                    for off in range(((w_i - 1) // 512) * 512, -1, -512):
                        w = min(512, w_i - off)
                        c = i * 128 + off
                        nc.tensor.matmul(
                            ps[:, off: off + w],
                            lhsT,
                            xv_s[k][:, :, c:c + w],
                            start=(k == 0), stop=(k == KTILES - 1),
                            perf_mode=PM.DoubleRow)
                sb = psim.tile([128, R], BF16, tag="simsb")
                nc.scalar.activation(sb[:, 0:w_i], ps[:, 0:w_i], AF.Copy)
                # block DMAs, grouped per (tile, chunk)
                g0 = _G0[i]
                g = g0
                while g < g0 + (NT - i):
                    q = 0
                    while g >= CH_CUM[q + 1]:
                        q += 1
                    hi = min(g0 + (NT - i), CH_CUM[q + 1])
                    nblk = hi - g
                    s0 = g - CH_CUM[q]
                    j0 = i + (g - g0)
                    dst = cc_tri_in[q][:].rearrange(
                        "(b p) c -> p b c", p=128)[:, s0:s0 + nblk, :]
                    src = sb[:, (j0 - i) * 128:(j0 - i + nblk) * 128]
                    nc.sync.dma_start(
                        dst, src.rearrange("p (b c) -> p b c", c=128))
                    g = hi

            # ---- chunked block ReduceScatter ----
            for q in range(NCH):
                nc.gpsimd.collective_compute(
                    "ReduceScatter", ALU.add, replica_groups=grp,
                    ins=[cc_tri_in[q][:].opt()], outs=[cc_tri_out[q][:].opt()])

            # ---- loss on owned blocks (exp/rowsum/pos/colsum per chunk) ----
            ptile = pers.tile([128, NSLOT], F32, tag="ptile")
            t_slot = 0
            for q in range(NCH):
                slab = pslab.tile([128, NB[q] * 128], BF16, tag="slab")
                nc.sync.dma_start(
                    slab[:].rearrange("p (b c) -> p b c", c=128),
                    cc_tri_out[q][:].rearrange("(b p) c -> p b c", p=128))
                E_q = []
                for l in range(NB[q]):
                    bt = slab[:, l * 128:(l + 1) * 128]
                    E_t = pers.tile([128, 128], BF16, tag=f"E{t_slot}")
                    rs_t = pers.tile([128, 1], F32, tag=f"rs{t_slot}")
                    nc.scalar.activation(E_t[:], bt, AF.Exp, scale=INV_T_S2,
                                         accum_out=rs_t[:])
                    E_q.append((t_slot, E_t))
                    # positives: (bt * pairflag) ⊙ eye, accumulated over free
                    scrE = pscr.tile([128, 128], BF16, tag="scrE")
                    nc.vector.scalar_tensor_tensor(
                        scrE[:], bt, pairsel_sb[:, t_slot:t_slot + 1],
                        eye_sb[:], ALU.mult, ALU.mult,
                        accum_out=ptile[:, t_slot:t_slot + 1])
                    # fold row sums into P_sb via selector mask
                    nc.vector.scalar_tensor_tensor(
                        P_sb[:], selrow_sb[:, t_slot * NT:(t_slot + 1) * NT],
                        rs_t[:, 0:1], P_sb[:], ALU.mult, ALU.add)
                    t_slot += 1
                # column sums on PE, one shared PSUM accumulation group
                if q == 0:
                    P_ps = pps.tile([128, NT], F32, tag="ps")
                for t, E_t in E_q:
                    nc.tensor.matmul(
                        P_ps[:], E_t[:],
                        selcol_sb[:, t * NT:(t + 1) * NT],
                        start=(t == 0), stop=(t == NSLOT - 1))

            # ---- tail ----
            if HOST_FINISH:
                # ship P and the positives partial; host does log + sums
                out_sb = pers.tile([128, NT + 1], F32, tag="outsb")
                scr17 = pers.tile([128, NSLOT], F32, tag="scr17")
                nc.vector.scalar_tensor_tensor(
                    scr17[:], ptile[:], 1.0, ptile[:], ALU.mult, ALU.max,
                    accum_out=out_sb[:, NT:NT + 1])
                nc.vector.tensor_tensor(out_sb[:, 0:NT], P_sb[:], P_ps[:],
                                        ALU.add)
                nc.sync.dma_start(y[:], out_sb[:])
            else:
                P_fin = pers.tile([128, NT], F32, tag="P_fin")
                nc.vector.tensor_tensor(P_fin[:], P_sb[:], P_ps[:], ALU.add)
                nc.sync.dma_start(cc_p_in[:], P_fin[:])
                nc.gpsimd.collective_compute(
                    "AllGather", ALU.bypass, replica_groups=grp,
                    ins=[cc_p_in[:].opt()], outs=[cc_p_out[:].opt()])
                pall_sb = pers.tile([128, NCORES * NT], F32, tag="pall")
                nc.sync.dma_start(
                    pall_sb[:].rearrange("p (b f) -> p b f", b=NCORES),
                    cc_p_out[:].rearrange("(b p) f -> p b f", p=128))
                Pa = pers.tile([128, NT], F32, tag="Pa")
                nc.vector.tensor_tensor(Pa[:], pall_sb[:, 0:NT],
                                        pall_sb[:, NT:2 * NT], ALU.add)
                for b in range(2, NCORES):
                    nc.vector.tensor_tensor(
                        Pa[:], Pa[:], pall_sb[:, b * NT:(b + 1) * NT], ALU.add)
                logP = pers.tile([128, NT], F32, tag="logP")
                nc.scalar.activation(logP[:], Pa[:], AF.Ln, bias=negE2[:, 0:1])
                lcol2 = pers.tile([128, 2], F32, tag="lcol2")
                scr16 = pers.tile([128, NT], F32, tag="scr16")
                nc.vector.scalar_tensor_tensor(
                    scr16[:], logP[:], 1.0, logmask_sb[:], ALU.mult, ALU.mult,
                    accum_out=lcol2[:, 0:1])
                scr17 = pers.tile([128, NSLOT], F32, tag="scr17")
                nc.vector.scalar_tensor_tensor(
                    scr17[:], ptile[:], 1.0, ptile[:], ALU.mult, ALU.max,
                    accum_out=lcol2[:, 1:2])
                loss_ps = pps.tile([1, 2], F32, tag="ps")
                nc.tensor.matmul(loss_ps[:], ones[:], lcol2[:],
                                 start=True, stop=True)
                out_sb = pers.tile([1, 2], F32, tag="outsb")
                nc.vector.tensor_copy(out_sb[:], loss_ps[:])
                nc.sync.dma_start(y[:], out_sb[:])

    nc.compile()
    _CACHE["nc"] = nc
    return nc


def _make_inputs(emb_i, emb_j):
    emb_i = np.asarray(emb_i, dtype=np.float32)
    emb_j = np.asarray(emb_j, dtype=np.float32)
    in_maps = []
    eye = np.eye(128, dtype=np.float32)
    xcs = []
    for c in range(NCORES):
        sl = slice(16 * c, 16 * (c + 1))
        xc = np.concatenate([emb_i[:, sl, :], emb_j[:, sl, :]], axis=0)
        # [r, m, n] -> [k, n, (s, r)] with m = 2k + s
        xc = xc.transpose(1, 2, 0).reshape(KTILES, 2, 128, R)
        xc = np.ascontiguousarray(xc.transpose(0, 2, 1, 3)).reshape(
            KTILES, 128, 2 * R).astype(ml_dtypes.float8_e4m3)
        xcs.append(xc)
    # per-(n, r) ssq over all m, from the fp8-quantized x (as the device saw it)
    ssq = np.zeros((128, R), dtype=np.float32)
    for c in range(NCORES):
        xf = xcs[c].astype(np.float32).reshape(KTILES, 128, 2, R)
        ssq += (xf * xf).sum(axis=(0, 2))
    scale8 = (S / np.sqrt(128.0 * np.maximum(ssq, 1e-24))).astype(
        ml_dtypes.float8_e4m3)

    for c in range(NCORES):
        slots = _core_slots(c)
        selrow = np.zeros((NSLOT, 128, NT), dtype=np.float32)
        selcol = np.zeros((NSLOT, 128, NT), dtype=np.float32)
        pairsel = np.zeros((128, NSLOT), dtype=np.float32)
        for t, g in enumerate(slots):
            i, j = BLOCKS[g]
            selrow[t, :, i] = 1.0
            if j != i:
                selcol[t, :, j] = 1.0
            if j == i + 8:
                pairsel[:, t] = INV_T_S2
        logmask = np.zeros((128, NT), dtype=np.float32)
        logmask[:, 2 * c] = 1.0
        logmask[:, 2 * c + 1] = 1.0
        in_maps.append({
            "x": xcs[c],
            "scale8": scale8,
            "selrow": np.ascontiguousarray(
                selrow.transpose(1, 0, 2).reshape(128, NSLOT * NT)
            ).astype(ml_dtypes.bfloat16),
            "selcol": np.ascontiguousarray(
                selcol.transpose(1, 0, 2).reshape(128, NSLOT * NT)
            ).astype(ml_dtypes.bfloat16),
            "pairsel": pairsel.astype(ml_dtypes.bfloat16),
            "eye": eye.astype(ml_dtypes.bfloat16),
            "logmask": logmask.astype(ml_dtypes.bfloat16),
        })
    return in_maps


def run(emb_i, emb_j, **spmd_kwargs):
    nc = _build_nc()
    in_maps = _make_inputs(emb_i, emb_j)
    res = bass_utils.run_bass_kernel_spmd(
        nc, in_maps, core_ids=list(range(NCORES)), **spmd_kwargs)
    if HOST_FINISH:
        P = np.zeros((128, NT), dtype=np.float64)
        pos = 0.0
        for r in res.results:
            yv = np.asarray(r["y"], dtype=np.float64)
            P += yv[:, 0:NT]
            pos += float(yv[:, NT].sum())
        total = float(np.log(P - E2).sum()) - 2.0 * pos
    else:
        total = sum(float(r["y"][0, 0]) - 2.0 * float(r["y"][0, 1])
                    for r in res.results)
    return np.array(total / R, dtype=np.float32), res


def kernel(emb_i, emb_j):
    loss, _ = run(emb_i, emb_j)
    return loss


# revision 17
# speedup vs baseline: 1.4467x; 1.4467x over previous
"""Contrastive loss (SimCLR-style, B=1024, emb [1024,128,128]) on 8 TRN2 cores.

Strategy (v4): K-sharded upper-triangle gram + block ReduceScatter.

Host prep per core c: x chunk (m-slice of 16) pre-transposed / fp8-quantized in
DoubleRow layout x[k, n, (s, r)] = fp8(emb[r, 16c + 2k + s, n]) exactly as the
data-parallel hint's local shard, plus the per-(n, r) normalization scale
scale8 = 64/sqrt(128*ssq) (fp8) and small constant selector masks.

Device per core:
  1. rn = x * scale8 in place (fp8) on DVE, hi r-half first then lo in two
     descending 512-col chunks, so the PE can start early and the wide row
     tiles unlock progressively.
  2. Upper-triangle gram: row tiles in order [15..8, 7..0], partial sim block
     row [128, (16-i)*128] f32 on PE (fp8 DoubleRow, K=256/instr) accumulated
     over the core's 8 K-tiles; PSUM -> bf16 (ACT copy) -> DRAM as [128,128]
     blocks in production order (136 blocks of the 16x16 upper triangle).
  3. Three block-aligned ReduceScatters (48+48+40 blocks) sum the partial
     blocks across cores; each core ends up owning 17 whole summed blocks.
  4. Loss: per owned block (i,j): E = exp(2*sim), ACT accum -> row sums
     (rows of tile i); PE matmul E^T @ onehot(j) -> column sums (rows of
     tile j, zero mask for i==j); positives from the diagonals of the 8
     pair blocks (i, i+8) via an eye-mask reduce. Per-core row-sum vector
     P [128,16] and positive partials are either finished on device (P
     AllGather + log) or shipped to the host (HOST_FINISH).
Host: loss = (sum_r log(sum_c P_c - e^2) - 2*sum_c pos_c) / 2048.
"""

import numpy as np
import ml_dtypes

import concourse.bacc as bacc
import concourse.mybir as mybir
import concourse.tile as tile
from concourse import bass_utils

F32 = mybir.dt.float32
BF16 = mybir.dt.bfloat16
FP8 = mybir.dt.float8e4
AF = mybir.ActivationFunctionType
ALU = mybir.AluOpType
PM = mybir.MatmulPerfMode

B = 1024
R = 2 * B            # 2048 rows
NCORES = 8
KTILES = 8           # DoubleRow K-tiles per core (256 K each)
NT = 16              # 128-row tiles of sim
S = 64.0             # fp8 prescale; sim comes out x S^2
INV_T_S2 = 2.0 / (S * S)   # 1/TEMP / S^2
E2 = float(np.exp(2.0))    # exp(self-sim / TEMP), exact constant
N_WARM = 16
HOST_FINISH = True

# Upper-triangle blocks in production order: small hi tiles first, then the
# wide tiles widest-last so ReduceScatter chunks materialize early.
PROD_TILES = list(range(NT - 1, 7, -1)) + list(range(0, 8))
BLOCKS = [(i, j) for i in PROD_TILES for j in range(i, NT)]   # 136
CH_SIZES = [48, 48, 40]
CH_CUM = [0, 48, 96, 136]
NB = [n // NCORES for n in CH_SIZES]          # owned blocks/chunk: [6, 6, 5]
NSLOT = sum(NB)                               # 17
NCH = len(CH_SIZES)

# global production index of block (i, i)
_G0 = {}
_g = 0
for _i in PROD_TILES:
    _G0[_i] = _g
    _g += NT - _i

_CACHE = {}


def _core_slots(c):
    """Global block ids owned by core c, in slot order."""
    out = []
    for q, nb in enumerate(NB):
        out.extend(range(CH_CUM[q] + c * nb, CH_CUM[q] + (c + 1) * nb))
    return out


def _build_nc():
    if "nc" in _CACHE:
        return _CACHE["nc"]
    nc = bacc.Bacc("TRN2", target_bir_lowering=False, debug=False,
                   num_devices=NCORES)

    x = nc.dram_tensor("x", [KTILES, 128, 2 * R], FP8, kind="ExternalInput")
    scale8_d = nc.dram_tensor("scale8", [128, R], FP8, kind="ExternalInput")
    selrow_d = nc.dram_tensor("selrow", [128, NSLOT * NT], BF16,
                              kind="ExternalInput")
    selcol_d = nc.dram_tensor("selcol", [128, NSLOT * NT], BF16,
                              kind="ExternalInput")
    pairsel_d = nc.dram_tensor("pairsel", [128, NSLOT], BF16,
                               kind="ExternalInput")
    eye_d = nc.dram_tensor("eye", [128, 128], BF16, kind="ExternalInput")
    logmask_d = nc.dram_tensor("logmask", [128, NT], BF16,
                               kind="ExternalInput")
    if HOST_FINISH:
        y = nc.dram_tensor("y", [128, NT + 1], F32, kind="ExternalOutput")
    else:
        y = nc.dram_tensor("y", [1, 2], F32, kind="ExternalOutput")

    cc_fl_in = nc.dram_tensor("cc_fl_in", [1, 128], BF16)
    cc_fl_out = nc.dram_tensor("cc_fl_out", [NCORES, 128], BF16,
                               addr_space="Shared")
    cc_fl_out2 = nc.dram_tensor("cc_fl_out2", [NCORES, 128], BF16,
                                addr_space="Shared")
    cc_tri_in = [nc.dram_tensor(f"cc_tri_in{q}", [CH_SIZES[q] * 128, 128],
                                BF16) for q in range(NCH)]
    cc_tri_out = [nc.dram_tensor(f"cc_tri_out{q}", [NB[q] * 128, 128], BF16)
                  for q in range(NCH)]
    cc_p_in = nc.dram_tensor("cc_p_in", [128, NT], F32)
    cc_p_out = nc.dram_tensor("cc_p_out", [NCORES * 128, NT], F32,
                              addr_space="Shared")
    grp = [list(range(NCORES))]

    with tile.TileContext(nc) as tc:
        with tc.tile_pool(name="x8", bufs=KTILES) as px8, \
             tc.tile_pool(name="simsb", bufs=4) as psim, \
             tc.tile_pool(name="slab", bufs=2) as pslab, \
             tc.tile_pool(name="scr", bufs=3) as pscr, \
             tc.tile_pool(name="pers", bufs=1) as pers, \
             tc.tile_pool(name="ps", bufs=2, space="PSUM") as pps:

            # ---- t0 DVE: warmup fodder + small constants ----
            junk8 = pers.tile([128, 512], FP8, tag="junk8")
            nc.vector.memset(junk8[:], 0.25)
            junkA = pers.tile([128, 16], F32, tag="junkA")
            nc.vector.memset(junkA[:], 1.0)
            ones = pers.tile([128, 1], F32, tag="ones")
            nc.vector.memset(ones[:], 1.0)
            P_sb = pers.tile([128, NT], F32, tag="P_sb")
            nc.vector.memset(P_sb[:], 0.0)
            negE2 = pers.tile([128, 1], F32, tag="negE2")
            nc.vector.memset(negE2[:], -E2)
            # ACT table preload: exp set covers exp/ln/copy/square
            junkB = pers.tile([128, 16], F32, tag="junkB")
            nc.scalar.activation(junkB[:], junkA[:], AF.Exp)

            # flush collective: absorbs the CC first-op launch overhead
            fl = pers.tile([1, 128], BF16, tag="fl")
            nc.gpsimd.memset(fl[:], 1.0)
            nc.gpsimd.dma_start(cc_fl_in[:], fl[:])
            nc.gpsimd.collective_compute(
                "AllGather", ALU.bypass, replica_groups=grp,
                ins=[cc_fl_in[:].opt()], outs=[cc_fl_out[:].opt()])

            # ---- PE warmup ----
            jv = junk8[:].rearrange("p (two n) -> p two n", two=2)
            ps_w = pps.tile([128, R], F32, tag="ps")
            for w in range(N_WARM):
                nc.tensor.matmul(ps_w[:, 0:256], jv[:, :, 0:128],
                                 jv[:, :, 0:256],
                                 start=(w == 0), stop=(w == N_WARM - 1),
                                 perf_mode=PM.DoubleRow)

            # ---- x DMAs: hi halves on SP, lo halves on GP queue ----
            xb = [px8.tile([128, 2 * R], FP8, tag="x8", name=f"xb{k}")
                  for k in range(KTILES)]
            xv_d = [x[k].rearrange("p (s r) -> p s r", s=2)
                    for k in range(KTILES)]
            xv_s = [xb[k][:].rearrange("p (s r) -> p s r", s=2)
                    for k in range(KTILES)]
            scale8 = pers.tile([128, R], FP8, tag="scale8")
            nc.sync.dma_start(scale8[:], scale8_d[:])
            for k in range(KTILES):
                nc.sync.dma_start(xv_s[k][:, :, B:R], xv_d[k][:, :, B:R])
            # lo halves on the ACT queue: keeps the gpsimd queue free so the
            # first collective triggers (and the CC barrier starts) at t~0
            for k in range(KTILES):
                nc.scalar.dma_start(xv_s[k][:, :, 0:B], xv_d[k][:, :, 0:B])

            # ---- masks on SP (contiguous, fast) ----
            selrow_sb = pers.tile([128, NSLOT * NT], BF16, tag="selrow")
            nc.sync.dma_start(selrow_sb[:], selrow_d[:])
            selcol_sb = pers.tile([128, NSLOT * NT], BF16, tag="selcol")
            nc.sync.dma_start(selcol_sb[:], selcol_d[:])
            pairsel_sb = pers.tile([128, NSLOT], BF16, tag="pairsel")
            nc.sync.dma_start(pairsel_sb[:], pairsel_d[:])
            eye_sb = pers.tile([128, 128], BF16, tag="eye")
            nc.sync.dma_start(eye_sb[:], eye_d[:])
            logmask_sb = pers.tile([128, NT], BF16, tag="logmask")
            nc.sync.dma_start(logmask_sb[:], logmask_d[:])

            # ---- normalize in place on DVE: hi half, then lo descending ----
            for k in range(KTILES):
                for s in range(2):
                    sl = xb[k][:, s * R + B: s * R + R]
                    nc.vector.tensor_tensor(sl, sl, scale8[:, B:R], ALU.mult)
            for c0 in (0, 512):
                for k in range(KTILES):
                    for s in range(2):
                        sl = xb[k][:, s * R + c0: s * R + c0 + 512]
                        nc.vector.tensor_tensor(sl, sl,
                                                scale8[:, c0:c0 + 512],
                                                ALU.mult)

            # ---- upper-triangle gram in production order ----
            for i in PROD_TILES:
                w_i = (NT - i) * 128
                ps = pps.tile([128, R], F32, tag="ps")
                for k in range(KTILES):
                    lhsT = xv_s[k][:, :, i * 128:(i + 1) * 128]
                    # chunks aligned to the 512-col PSUM bank grid (a matmul
                    # dst must not cross a bank boundary); descending so the
                    # hi r-half is consumed first
                    for off in range(((w_i - 1) // 512) * 512, -1, -512):
                        w = min(512, w_i - off)
                        c = i * 128 + off
                        nc.tensor.matmul(
                            ps[:, off: off + w],
                            lhsT,
                            xv_s[k][:, :, c:c + w],
                            start=(k == 0), stop=(k == KTILES - 1),
                            perf_mode=PM.DoubleRow)
                sb = psim.tile([128, R], BF16, tag="simsb")
                nc.scalar.activation(sb[:, 0:w_i], ps[:, 0:w_i], AF.Copy)
                # block DMAs, grouped per (tile, chunk)
                g0 = _G0[i]
                g = g0
                while g < g0 + (NT - i):
                    q = 0
                    while g >= CH_CUM[q + 1]:
                        q += 1
                    hi = min(g0 + (NT - i), CH_CUM[q + 1])
                    nblk = hi - g
                    s0 = g - CH_CUM[q]
                    j0 = i + (g - g0)
                    dst = cc_tri_in[q][:].rearrange(
                        "(b p) c -> p b c", p=128)[:, s0:s0 + nblk, :]
                    src = sb[:, (j0 - i) * 128:(j0 - i + nblk) * 128]
                    nc.sync.dma_start(
                        dst, src.rearrange("p (b c) -> p b c", c=128))
                    g = hi

            # ---- chunked block ReduceScatter ----
            for q in range(NCH):
                nc.gpsimd.collective_compute(
                    "ReduceScatter", ALU.add, replica_groups=grp,
                    ins=[cc_tri_in[q][:].opt()], outs=[cc_tri_out[q][:].opt()])

            # ---- loss on owned blocks (exp/rowsum/pos/colsum per chunk) ----
            ptile = pers.tile([128, NSLOT], F32, tag="ptile")
            t_slot = 0
            for q in range(NCH):
                slab = pslab.tile([128, NB[q] * 128], BF16, tag="slab")
                nc.sync.dma_start(
                    slab[:].rearrange("p (b c) -> p b c", c=128),
                    cc_tri_out[q][:].rearrange("(b p) c -> p b c", p=128))
                E_q = []
                for l in range(NB[q]):
                    bt = slab[:, l * 128:(l + 1) * 128]
                    E_t = pers.tile([128, 128], BF16, tag=f"E{t_slot}")
                    rs_t = pers.tile([128, 1], F32, tag=f"rs{t_slot}")
                    nc.scalar.activation(E_t[:], bt, AF.Exp, scale=INV_T_S2,
                                         accum_out=rs_t[:])
                    E_q.append((t_slot, E_t))
                    # positives: (bt * pairflag) ⊙ eye, accumulated over free
                    scrE = pscr.tile([128, 128], BF16, tag="scrE")
                    nc.vector.scalar_tensor_tensor(
                        scrE[:], bt, pairsel_sb[:, t_slot:t_slot + 1],
                        eye_sb[:], ALU.mult, ALU.mult,
                        accum_out=ptile[:, t_slot:t_slot + 1])
                    # fold row sums into P_sb via selector mask
                    nc.vector.scalar_tensor_tensor(
                        P_sb[:], selrow_sb[:, t_slot * NT:(t_slot + 1) * NT],
                        rs_t[:, 0:1], P_sb[:], ALU.mult, ALU.add)
                    t_slot += 1
                # column sums on PE, one shared PSUM accumulation group
                if q == 0:
                    P_ps = pps.tile([128, NT], F32, tag="ps")
                for t, E_t in E_q:
                    nc.tensor.matmul(
                        P_ps[:], E_t[:],
                        selcol_sb[:, t * NT:(t + 1) * NT],
                        start=(t == 0), stop=(t == NSLOT - 1))

            # ---- tail ----
            if HOST_FINISH:
                # ship P and the positives partial; host does log + sums
                out_sb = pers.tile([128, NT + 1], F32, tag="outsb")
                scr17 = pers.tile([128, NSLOT], F32, tag="scr17")
                nc.vector.scalar_tensor_tensor(
                    scr17[:], ptile[:], 1.0, ptile[:], ALU.mult, ALU.max,
                    accum_out=out_sb[:, NT:NT + 1])
                nc.vector.tensor_tensor(out_sb[:, 0:NT], P_sb[:], P_ps[:],
                                        ALU.add)
                nc.sync.dma_start(y[:], out_sb[:])
            else:
                P_fin = pers.tile([128, NT], F32, tag="P_fin")
                nc.vector.tensor_tensor(P_fin[:], P_sb[:], P_ps[:], ALU.add)
                nc.sync.dma_start(cc_p_in[:], P_fin[:])
                nc.gpsimd.collective_compute(
                    "AllGather", ALU.bypass, replica_groups=grp,
                    ins=[cc_p_in[:].opt()], outs=[cc_p_out[:].opt()])
                pall_sb = pers.tile([128, NCORES * NT], F32, tag="pall")
                nc.sync.dma_start(
                    pall_sb[:].rearrange("p (b f) -> p b f", b=NCORES),
                    cc_p_out[:].rearrange("(b p) f -> p b f", p=128))
                Pa = pers.tile([128, NT], F32, tag="Pa")
                nc.vector.tensor_tensor(Pa[:], pall_sb[:, 0:NT],
                                        pall_sb[:, NT:2 * NT], ALU.add)
                for b in range(2, NCORES):
                    nc.vector.tensor_tensor(
                        Pa[:], Pa[:], pall_sb[:, b * NT:(b + 1) * NT], ALU.add)
                logP = pers.tile([128, NT], F32, tag="logP")
                nc.scalar.activation(logP[:], Pa[:], AF.Ln, bias=negE2[:, 0:1])
                lcol2 = pers.tile([128, 2], F32, tag="lcol2")
                scr16 = pers.tile([128, NT], F32, tag="scr16")
                nc.vector.scalar_tensor_tensor(
                    scr16[:], logP[:], 1.0, logmask_sb[:], ALU.mult, ALU.mult,
                    accum_out=lcol2[:, 0:1])
                scr17 = pers.tile([128, NSLOT], F32, tag="scr17")
                nc.vector.scalar_tensor_tensor(
                    scr17[:], ptile[:], 1.0, ptile[:], ALU.mult, ALU.max,
                    accum_out=lcol2[:, 1:2])
                loss_ps = pps.tile([1, 2], F32, tag="ps")
                nc.tensor.matmul(loss_ps[:], ones[:], lcol2[:],
                                 start=True, stop=True)
                out_sb = pers.tile([1, 2], F32, tag="outsb")
                nc.vector.tensor_copy(out_sb[:], loss_ps[:])
                nc.sync.dma_start(y[:], out_sb[:])

    nc.compile()
    _CACHE["nc"] = nc
    return nc


def _make_inputs(emb_i, emb_j):
    emb_i = np.asarray(emb_i, dtype=np.float32)
    emb_j = np.asarray(emb_j, dtype=np.float32)
    in_maps = []
    eye = np.eye(128, dtype=np.float32)
    xcs = []
    for c in range(NCORES):
        sl = slice(16 * c, 16 * (c + 1))
        xc = np.concatenate([emb_i[:, sl, :], emb_j[:, sl, :]], axis=0)
        # [r, m, n] -> [k, n, (s, r)] with m = 2k + s
        xc = xc.transpose(1, 2, 0).reshape(KTILES, 2, 128, R)
        xc = np.ascontiguousarray(xc.transpose(0, 2, 1, 3)).reshape(
            KTILES, 128, 2 * R).astype(ml_dtypes.float8_e4m3)
        xcs.append(xc)
    # per-(n, r) ssq over all m, from the fp8-quantized x (as the device saw it)
    ssq = np.zeros((128, R), dtype=np.float32)
    for c in range(NCORES):
        xf = xcs[c].astype(np.float32).reshape(KTILES, 128, 2, R)
        ssq += (xf * xf).sum(axis=(0, 2))
    scale8 = (S / np.sqrt(128.0 * np.maximum(ssq, 1e-24))).astype(
        ml_dtypes.float8_e4m3)

    for c in range(NCORES):
        slots = _core_slots(c)
        selrow = np.zeros((NSLOT, 128, NT), dtype=np.float32)
        selcol = np.zeros((NSLOT, 128, NT), dtype=np.float32)
        pairsel = np.zeros((128, NSLOT), dtype=np.float32)
        for t, g in enumerate(slots):
            i, j = BLOCKS[g]
            selrow[t, :, i] = 1.0
            if j != i:
                selcol[t, :, j] = 1.0
            if j == i + 8:
                pairsel[:, t] = INV_T_S2
        logmask = np.zeros((128, NT), dtype=np.float32)
        logmask[:, 2 * c] = 1.0
        logmask[:, 2 * c + 1] = 1.0
        in_maps.append({
            "x": xcs[c],
            "scale8": scale8,
            "selrow": np.ascontiguousarray(
                selrow.transpose(1, 0, 2).reshape(128, NSLOT * NT)
            ).astype(ml_dtypes.bfloat16),
            "selcol": np.ascontiguousarray(
                selcol.transpose(1, 0, 2).reshape(128, NSLOT * NT)
            ).astype(ml_dtypes.bfloat16),
            "pairsel": pairsel.astype(ml_dtypes.bfloat16),
            "eye": eye.astype(ml_dtypes.bfloat16),
            "logmask": logmask.astype(ml_dtypes.bfloat16),
        })
    return in_maps


def run(emb_i, emb_j, **spmd_kwargs):
    nc = _build_nc()
    in_maps = _make_inputs(emb_i, emb_j)
    res = bass_utils.run_bass_kernel_spmd(
        nc, in_maps, core_ids=list(range(NCORES)), **spmd_kwargs)
    if HOST_FINISH:
        P = np.zeros((128, NT), dtype=np.float64)
        pos = 0.0
        for r in res.results:
            yv = np.asarray(r["y"], dtype=np.float64)
            P += yv[:, 0:NT]
            pos += float(yv[:, NT].sum())
        total = float(np.log(P - E2).sum()) - 2.0 * pos
    else:
        total = sum(float(r["y"][0, 0]) - 2.0 * float(r["y"][0, 1])
                    for r in res.results)
    return np.array(total / R, dtype=np.float32), res


def kernel(emb_i, emb_j):
    loss, _ = run(emb_i, emb_j)
    return loss


# revision 18
# speedup vs baseline: 1.6713x; 1.1553x over previous
"""Contrastive loss (SimCLR-style, B=1024, emb [1024,128,128]) on 8 TRN2 cores.

Strategy (v4): K-sharded upper-triangle gram + block ReduceScatter.

Host prep per core c: x chunk (m-slice of 16) pre-transposed / fp8-quantized in
DoubleRow layout x[k, n, (s, r)] = fp8(emb[r, 16c + 2k + s, n]) exactly as the
data-parallel hint's local shard, plus the per-(n, r) normalization scale
scale8 = 64/sqrt(128*ssq) (fp8) and small constant selector masks.

Device per core:
  1. rn = x * scale8 in place (fp8) on DVE, hi r-half first then lo in two
     descending 512-col chunks, so the PE can start early and the wide row
     tiles unlock progressively.
  2. Upper-triangle gram: row tiles in order [15..8, 7..0], partial sim block
     row [128, (16-i)*128] f32 on PE (fp8 DoubleRow, K=256/instr) accumulated
     over the core's 8 K-tiles; PSUM -> bf16 (ACT copy) -> DRAM as [128,128]
     blocks in production order (136 blocks of the 16x16 upper triangle).
  3. Three block-aligned ReduceScatters (48+48+40 blocks) sum the partial
     blocks across cores; each core ends up owning 17 whole summed blocks.
  4. Loss: per owned block (i,j): E = exp(2*sim), ACT accum -> row sums
     (rows of tile i); PE matmul E^T @ onehot(j) -> column sums (rows of
     tile j, zero mask for i==j); positives from the diagonals of the 8
     pair blocks (i, i+8) via an eye-mask reduce. Per-core row-sum vector
     P [128,16] and positive partials are either finished on device (P
     AllGather + log) or shipped to the host (HOST_FINISH).
Host: loss = (sum_r log(sum_c P_c - e^2) - 2*sum_c pos_c) / 2048.
"""

import numpy as np
import ml_dtypes

import concourse.bacc as bacc
import concourse.mybir as mybir
import concourse.tile as tile
from concourse import bass_utils

F32 = mybir.dt.float32
BF16 = mybir.dt.bfloat16
FP8 = mybir.dt.float8e4
AF = mybir.ActivationFunctionType
ALU = mybir.AluOpType
PM = mybir.MatmulPerfMode

B = 1024
R = 2 * B            # 2048 rows
NCORES = 8
KTILES = 8           # DoubleRow K-tiles per core (256 K each)
NT = 16              # 128-row tiles of sim
S = 64.0             # fp8 prescale; sim comes out x S^2
INV_T_S2 = 2.0 / (S * S)   # 1/TEMP / S^2
E2 = float(np.exp(2.0))    # exp(self-sim / TEMP), exact constant
N_WARM = 16
HOST_FINISH = True

# Upper-triangle blocks in production order: small hi tiles first, then the
# wide tiles widest-last so ReduceScatter chunks materialize early.
PROD_TILES = list(range(NT - 1, 7, -1)) + list(range(7, -1, -1))
BLOCKS = [(i, j) for i in PROD_TILES for j in range(i, NT)]   # 136
CH_SIZES = [48, 48, 40]
CH_CUM = [0, 48, 96, 136]
NB = [n // NCORES for n in CH_SIZES]          # owned blocks/chunk: [6, 6, 5]
NSLOT = sum(NB)                               # 17
NCH = len(CH_SIZES)

# global production index of block (i, i)
_G0 = {}
_g = 0
for _i in PROD_TILES:
    _G0[_i] = _g
    _g += NT - _i

_CACHE = {}


def _core_slots(c):
    """Global block ids owned by core c, in slot order."""
    out = []
    for q, nb in enumerate(NB):
        out.extend(range(CH_CUM[q] + c * nb, CH_CUM[q] + (c + 1) * nb))
    return out


def _build_nc():
    if "nc" in _CACHE:
        return _CACHE["nc"]
    nc = bacc.Bacc("TRN2", target_bir_lowering=False, debug=False,
                   num_devices=NCORES)

    x = nc.dram_tensor("x", [KTILES, 128, 2 * R], FP8, kind="ExternalInput")
    scale8_d = nc.dram_tensor("scale8", [128, R], FP8, kind="ExternalInput")
    selrow_d = nc.dram_tensor("selrow", [128, NSLOT * NT], BF16,
                              kind="ExternalInput")
    selcol_d = nc.dram_tensor("selcol", [128, NSLOT * NT], BF16,
                              kind="ExternalInput")
    pairsel_d = nc.dram_tensor("pairsel", [128, NSLOT], BF16,
                               kind="ExternalInput")
    eye_d = nc.dram_tensor("eye", [128, 128], BF16, kind="ExternalInput")
    logmask_d = nc.dram_tensor("logmask", [128, NT], BF16,
                               kind="ExternalInput")
    if HOST_FINISH:
        y = nc.dram_tensor("y", [128, NT + 1], F32, kind="ExternalOutput")
    else:
        y = nc.dram_tensor("y", [1, 2], F32, kind="ExternalOutput")

    cc_fl_in = nc.dram_tensor("cc_fl_in", [1, 128], BF16)
    cc_fl_out = nc.dram_tensor("cc_fl_out", [NCORES, 128], BF16,
                               addr_space="Shared")
    cc_fl_out2 = nc.dram_tensor("cc_fl_out2", [NCORES, 128], BF16,
                                addr_space="Shared")
    cc_tri_in = [nc.dram_tensor(f"cc_tri_in{q}", [CH_SIZES[q] * 128, 128],
                                BF16) for q in range(NCH)]
    cc_tri_out = [nc.dram_tensor(f"cc_tri_out{q}", [NB[q] * 128, 128], BF16)
                  for q in range(NCH)]
    cc_p_in = nc.dram_tensor("cc_p_in", [128, NT], F32)
    cc_p_out = nc.dram_tensor("cc_p_out", [NCORES * 128, NT], F32,
                              addr_space="Shared")
    grp = [list(range(NCORES))]

    with tile.TileContext(nc) as tc:
        with tc.tile_pool(name="x8", bufs=KTILES) as px8, \
             tc.tile_pool(name="simsb", bufs=4) as psim, \
             tc.tile_pool(name="slab", bufs=2) as pslab, \
             tc.tile_pool(name="scr", bufs=3) as pscr, \
             tc.tile_pool(name="pers", bufs=1) as pers, \
             tc.tile_pool(name="ps", bufs=2, space="PSUM") as pps:

            # ---- t0 DVE: warmup fodder + small constants ----
            junk8 = pers.tile([128, 512], FP8, tag="junk8")
            nc.vector.memset(junk8[:], 0.25)
            junkA = pers.tile([128, 16], F32, tag="junkA")
            nc.vector.memset(junkA[:], 1.0)
            ones = pers.tile([128, 1], F32, tag="ones")
            nc.vector.memset(ones[:], 1.0)
            P_sb = pers.tile([128, NT], F32, tag="P_sb")
            nc.vector.memset(P_sb[:], 0.0)
            negE2 = pers.tile([128, 1], F32, tag="negE2")
            nc.vector.memset(negE2[:], -E2)
            # ACT table preload: exp set covers exp/ln/copy/square
            junkB = pers.tile([128, 16], F32, tag="junkB")
            nc.scalar.activation(junkB[:], junkA[:], AF.Exp)

            # flush collective: absorbs the CC first-op launch overhead
            fl = pers.tile([1, 128], BF16, tag="fl")
            nc.gpsimd.memset(fl[:], 1.0)
            nc.gpsimd.dma_start(cc_fl_in[:], fl[:])
            nc.gpsimd.collective_compute(
                "AllGather", ALU.bypass, replica_groups=grp,
                ins=[cc_fl_in[:].opt()], outs=[cc_fl_out[:].opt()])

            # ---- PE warmup ----
            jv = junk8[:].rearrange("p (two n) -> p two n", two=2)
            ps_w = pps.tile([128, R], F32, tag="ps")
            for w in range(N_WARM):
                nc.tensor.matmul(ps_w[:, 0:256], jv[:, :, 0:128],
                                 jv[:, :, 0:256],
                                 start=(w == 0), stop=(w == N_WARM - 1),
                                 perf_mode=PM.DoubleRow)

            # ---- x DMAs: hi halves on SP, lo halves on GP queue ----
            xb = [px8.tile([128, 2 * R], FP8, tag="x8", name=f"xb{k}")
                  for k in range(KTILES)]
            xv_d = [x[k].rearrange("p (s r) -> p s r", s=2)
                    for k in range(KTILES)]
            xv_s = [xb[k][:].rearrange("p (s r) -> p s r", s=2)
                    for k in range(KTILES)]
            scale8 = pers.tile([128, R], FP8, tag="scale8")
            nc.sync.dma_start(scale8[:], scale8_d[:])
            for k in range(KTILES):
                nc.sync.dma_start(xv_s[k][:, :, B:R], xv_d[k][:, :, B:R])
            # lo halves on the ACT queue: keeps the gpsimd queue free so the
            # first collective triggers (and the CC barrier starts) at t~0
            for k in range(KTILES):
                nc.scalar.dma_start(xv_s[k][:, :, 0:B], xv_d[k][:, :, 0:B])

            # ---- masks on SP (contiguous, fast) ----
            selrow_sb = pers.tile([128, NSLOT * NT], BF16, tag="selrow")
            nc.sync.dma_start(selrow_sb[:], selrow_d[:])
            selcol_sb = pers.tile([128, NSLOT * NT], BF16, tag="selcol")
            nc.sync.dma_start(selcol_sb[:], selcol_d[:])
            pairsel_sb = pers.tile([128, NSLOT], BF16, tag="pairsel")
            nc.sync.dma_start(pairsel_sb[:], pairsel_d[:])
            eye_sb = pers.tile([128, 128], BF16, tag="eye")
            nc.sync.dma_start(eye_sb[:], eye_d[:])
            logmask_sb = pers.tile([128, NT], BF16, tag="logmask")
            nc.sync.dma_start(logmask_sb[:], logmask_d[:])

            # ---- normalize in place on DVE: hi half, then lo descending ----
            for k in range(KTILES):
                for s in range(2):
                    sl = xb[k][:, s * R + B: s * R + R]
                    nc.vector.tensor_tensor(sl, sl, scale8[:, B:R], ALU.mult)
            for c0 in (512, 0):
                for k in range(KTILES):
                    for s in range(2):
                        sl = xb[k][:, s * R + c0: s * R + c0 + 512]
                        nc.vector.tensor_tensor(sl, sl,
                                                scale8[:, c0:c0 + 512],
                                                ALU.mult)

            # ---- upper-triangle gram in production order ----
            for i in PROD_TILES:
                w_i = (NT - i) * 128
                ps = pps.tile([128, R], F32, tag="ps")
                for k in range(KTILES):
                    lhsT = xv_s[k][:, :, i * 128:(i + 1) * 128]
                    # chunks aligned to the 512-col PSUM bank grid (a matmul
                    # dst must not cross a bank boundary); descending so the
                    # hi r-half is consumed first
                    for off in range(((w_i - 1) // 512) * 512, -1, -512):
                        w = min(512, w_i - off)
                        c = i * 128 + off
                        nc.tensor.matmul(
                            ps[:, off: off + w],
                            lhsT,
                            xv_s[k][:, :, c:c + w],
                            start=(k == 0), stop=(k == KTILES - 1),
                            perf_mode=PM.DoubleRow)
                sb = psim.tile([128, R], BF16, tag="simsb")
                nc.scalar.activation(sb[:, 0:w_i], ps[:, 0:w_i], AF.Copy)
                # block DMAs, grouped per (tile, chunk)
                g0 = _G0[i]
                g = g0
                while g < g0 + (NT - i):
                    q = 0
                    while g >= CH_CUM[q + 1]:
                        q += 1
                    hi = min(g0 + (NT - i), CH_CUM[q + 1])
                    nblk = hi - g
                    s0 = g - CH_CUM[q]
                    j0 = i + (g - g0)
                    dst = cc_tri_in[q][:].rearrange(
                        "(b p) c -> p b c", p=128)[:, s0:s0 + nblk, :]
                    src = sb[:, (j0 - i) * 128:(j0 - i + nblk) * 128]
                    nc.sync.dma_start(
                        dst, src.rearrange("p (b c) -> p b c", c=128))
                    g = hi

            # ---- chunked block ReduceScatter ----
            for q in range(NCH):
                nc.gpsimd.collective_compute(
                    "ReduceScatter", ALU.add, replica_groups=grp,
                    ins=[cc_tri_in[q][:].opt()], outs=[cc_tri_out[q][:].opt()])

            # ---- loss on owned blocks (exp/rowsum/pos/colsum per chunk) ----
            ptile = pers.tile([128, NSLOT], F32, tag="ptile")
            t_slot = 0
            for q in range(NCH):
                slab = pslab.tile([128, NB[q] * 128], BF16, tag="slab")
                nc.sync.dma_start(
                    slab[:].rearrange("p (b c) -> p b c", c=128),
                    cc_tri_out[q][:].rearrange("(b p) c -> p b c", p=128))
                E_q = []
                for l in range(NB[q]):
                    bt = slab[:, l * 128:(l + 1) * 128]
                    E_t = pers.tile([128, 128], BF16, tag=f"E{t_slot}")
                    rs_t = pers.tile([128, 1], F32, tag=f"rs{t_slot}")
                    nc.scalar.activation(E_t[:], bt, AF.Exp, scale=INV_T_S2,
                                         accum_out=rs_t[:])
                    E_q.append((t_slot, E_t))
                    # positives: (bt * pairflag) ⊙ eye, accumulated over free
                    scrE = pscr.tile([128, 128], BF16, tag="scrE")
                    nc.vector.scalar_tensor_tensor(
                        scrE[:], bt, pairsel_sb[:, t_slot:t_slot + 1],
                        eye_sb[:], ALU.mult, ALU.mult,
                        accum_out=ptile[:, t_slot:t_slot + 1])
                    # fold row sums into P_sb via selector mask
                    nc.vector.scalar_tensor_tensor(
                        P_sb[:], selrow_sb[:, t_slot * NT:(t_slot + 1) * NT],
                        rs_t[:, 0:1], P_sb[:], ALU.mult, ALU.add)
                    t_slot += 1
                # column sums on PE, one shared PSUM accumulation group
                if q == 0:
                    P_ps = pps.tile([128, NT], F32, tag="ps")
                for t, E_t in E_q:
                    nc.tensor.matmul(
                        P_ps[:], E_t[:],
                        selcol_sb[:, t * NT:(t + 1) * NT],
                        start=(t == 0), stop=(t == NSLOT - 1))

            # ---- tail ----
            if HOST_FINISH:
                # ship P and the positives partial; host does log + sums
                out_sb = pers.tile([128, NT + 1], F32, tag="outsb")
                scr17 = pers.tile([128, NSLOT], F32, tag="scr17")
                nc.vector.scalar_tensor_tensor(
                    scr17[:], ptile[:], 1.0, ptile[:], ALU.mult, ALU.max,
                    accum_out=out_sb[:, NT:NT + 1])
                nc.vector.tensor_tensor(out_sb[:, 0:NT], P_sb[:], P_ps[:],
                                        ALU.add)
                nc.sync.dma_start(y[:], out_sb[:])
            else:
                P_fin = pers.tile([128, NT], F32, tag="P_fin")
                nc.vector.tensor_tensor(P_fin[:], P_sb[:], P_ps[:], ALU.add)
                nc.sync.dma_start(cc_p_in[:], P_fin[:])
                nc.gpsimd.collective_compute(
                    "AllGather", ALU.bypass, replica_groups=grp,
                    ins=[cc_p_in[:].opt()], outs=[cc_p_out[:].opt()])
                pall_sb = pers.tile([128, NCORES * NT], F32, tag="pall")
                nc.sync.dma_start(
                    pall_sb[:].rearrange("p (b f) -> p b f", b=NCORES),
                    cc_p_out[:].rearrange("(b p) f -> p b f", p=128))
                Pa = pers.tile([128, NT], F32, tag="Pa")
                nc.vector.tensor_tensor(Pa[:], pall_sb[:, 0:NT],
                                        pall_sb[:, NT:2 * NT], ALU.add)
                for b in range(2, NCORES):
                    nc.vector.tensor_tensor(
                        Pa[:], Pa[:], pall_sb[:, b * NT:(b + 1) * NT], ALU.add)
                logP = pers.tile([128, NT], F32, tag="logP")
                nc.scalar.activation(logP[:], Pa[:], AF.Ln, bias=negE2[:, 0:1])
                lcol2 = pers.tile([128, 2], F32, tag="lcol2")
                scr16 = pers.tile([128, NT], F32, tag="scr16")
                nc.vector.scalar_tensor_tensor(
                    scr16[:], logP[:], 1.0, logmask_sb[:], ALU.mult, ALU.mult,
                    accum_out=lcol2[:, 0:1])
                scr17 = pers.tile([128, NSLOT], F32, tag="scr17")
                nc.vector.scalar_tensor_tensor(
                    scr17[:], ptile[:], 1.0, ptile[:], ALU.mult, ALU.max,
                    accum_out=lcol2[:, 1:2])
                loss_ps = pps.tile([1, 2], F32, tag="ps")
                nc.tensor.matmul(loss_ps[:], ones[:], lcol2[:],
                                 start=True, stop=True)
                out_sb = pers.tile([1, 2], F32, tag="outsb")
                nc.vector.tensor_copy(out_sb[:], loss_ps[:])
                nc.sync.dma_start(y[:], out_sb[:])

    nc.compile()
    _CACHE["nc"] = nc
    return nc


def _make_inputs(emb_i, emb_j):
    emb_i = np.asarray(emb_i, dtype=np.float32)
    emb_j = np.asarray(emb_j, dtype=np.float32)
    in_maps = []
    eye = np.eye(128, dtype=np.float32)
    xcs = []
    for c in range(NCORES):
        sl = slice(16 * c, 16 * (c + 1))
        xc = np.concatenate([emb_i[:, sl, :], emb_j[:, sl, :]], axis=0)
        # [r, m, n] -> [k, n, (s, r)] with m = 2k + s
        xc = xc.transpose(1, 2, 0).reshape(KTILES, 2, 128, R)
        xc = np.ascontiguousarray(xc.transpose(0, 2, 1, 3)).reshape(
            KTILES, 128, 2 * R).astype(ml_dtypes.float8_e4m3)
        xcs.append(xc)
    # per-(n, r) ssq over all m, from the fp8-quantized x (as the device saw it)
    ssq = np.zeros((128, R), dtype=np.float32)
    for c in range(NCORES):
        xf = xcs[c].astype(np.float32).reshape(KTILES, 128, 2, R)
        ssq += (xf * xf).sum(axis=(0, 2))
    scale8 = (S / np.sqrt(128.0 * np.maximum(ssq, 1e-24))).astype(
        ml_dtypes.float8_e4m3)

    for c in range(NCORES):
        slots = _core_slots(c)
        selrow = np.zeros((NSLOT, 128, NT), dtype=np.float32)
        selcol = np.zeros((NSLOT, 128, NT), dtype=np.float32)
        pairsel = np.zeros((128, NSLOT), dtype=np.float32)
        for t, g in enumerate(slots):
            i, j = BLOCKS[g]
            selrow[t, :, i] = 1.0
            if j != i:
                selcol[t, :, j] = 1.0
            if j == i + 8:
                pairsel[:, t] = INV_T_S2
        logmask = np.zeros((128, NT), dtype=np.float32)
        logmask[:, 2 * c] = 1.0
        logmask[:, 2 * c + 1] = 1.0
        in_maps.append({
            "x": xcs[c],
            "scale8": scale8,
            "selrow": np.ascontiguousarray(
                selrow.transpose(1, 0, 2).reshape(128, NSLOT * NT)
            ).astype(ml_dtypes.bfloat16),
            "selcol": np.ascontiguousarray(
                selcol.transpose(1, 0, 2).reshape(128, NSLOT * NT)
            ).astype(ml_dtypes.bfloat16),
            "pairsel": pairsel.astype(ml_dtypes.bfloat16),
            "eye": eye.astype(ml_dtypes.bfloat16),
            "logmask": logmask.astype(ml_dtypes.bfloat16),
        })
    return in_maps


def run(emb_i, emb_j, **spmd_kwargs):
    nc = _build_nc()
    in_maps = _make_inputs(emb_i, emb_j)
    res = bass_utils.run_bass_kernel_spmd(
        nc, in_maps, core_ids=list(range(NCORES)), **spmd_kwargs)
    if HOST_FINISH:
        P = np.zeros((128, NT), dtype=np.float64)
        pos = 0.0
        for r in res.results:
            yv = np.asarray(r["y"], dtype=np.float64)
            P += yv[:, 0:NT]
            pos += float(yv[:, NT].sum())
        total = float(np.log(P - E2).sum()) - 2.0 * pos
    else:
        total = sum(float(r["y"][0, 0]) - 2.0 * float(r["y"][0, 1])
                    for r in res.results)
    return np.array(total / R, dtype=np.float32), res


def kernel(emb_i, emb_j):
    loss, _ = run(emb_i, emb_j)
    return loss
